# revision 13
# baseline (speedup 1.0000x reference)
"""Trainium2 Bass kernel for nn_MGCNLoss (segment_reduce).

Strategy (8 NeuronCores, SPMD):
  * Graph-sharded data parallelism: core c owns graphs [512c, 512(c+1)).
  * Host-side sharding step routes every node to its owning core and lays the
    core's nodes out as a fixed-stride padded matrix [512 graphs, PAD slots]
    (zero padding; PAD=2304 >= max nodes/graph). With that layout the on-device
    segment_sum is a dense per-partition row reduction (partition p of
    supertile s holds graph 512c+128s+p), the per-node normalization
    score/(sum[batch]+eps) is a per-partition broadcast, and the whole kernel
    is memory/DVE-bound as the problem's target_regime intends.
  * Device computes, per core: per-graph sums (segment_sum partials), their
    reciprocals, the per-node JS/KL terms (via ACT Ln + DVE fused
    multiply-accumulate), per-graph cross-entropy (max/exp/sum/log-softmax +
    one-hot target pick) and the correlation MSE, reduced to per-partition
    partials; partials are all-reduced across the 8 cores with a collective
    and every core computes the identical final (l_total, l_train, l_cor).

KL identity used (exactly the reference math, no approximation):
    sum_i [s_p*log((s_p+e)/(m+e)) + s_n*log((s_n+e)/(m+e))]
  = sum_i [s_p*Lp + s_n*Ln - (s_p+s_n)*Lm]
  with Lp=log(s_p+e), Ln=log(s_n+e), Lm=log(0.5*(s_p+s_n)+e)
  and sum_i s_p*Lp = r_p * sum_i x_i*Lp  (r_p is constant per graph/partition).
"""

import os

import numpy as np

import concourse.bass as bass
import concourse.bacc as bacc
import concourse.mybir as mybir
from concourse import tile
from concourse.bass_utils import run_bass_kernel_spmd

F32 = mybir.dt.float32
ALU = mybir.AluOpType
ACTF = mybir.ActivationFunctionType
AX = mybir.AxisListType

NUM_GRAPHS = 4096
NUM_NODES = 8_388_608
NUM_CLASSES = 10
NCORES = 8
GPC = NUM_GRAPHS // NCORES  # graphs per core = 512
ST = GPC // 128  # supertiles per core = 4
PAD = 2304  # padded slots per graph (actual max graph size is 2229)
NCH = 2  # chunks per supertile for pass 2
EPS = 1e-8
ALPHA = 1.0
BETA = 1.0
LAMBDA_COR = 0.1

LAST_RESULTS = None  # BassKernelResults of the most recent run (for test harness)


def _build_nc(pad: int, nch: int) -> bass.Bass:
    """Build the SPMD Bass program (identical on all 8 cores)."""
    ch = pad // nch
    assert ch * nch == pad
    nc = bacc.Bacc(None, num_devices=NCORES)

    xp_d = nc.declare_dram_parameter("xp", [ST, 128, pad], F32, isOutput=False)
    xn_d = nc.declare_dram_parameter("xn", [ST, 128, pad], F32, isOutput=False)
    # meta: per graph row: [0:10]=logits, [10:20]=probs_pos, [20:30]=probs_neg,
    # [30]=target (as f32), [31]=zero pad
    mt_d = nc.declare_dram_parameter("mt", [ST, 128, 32], F32, isOutput=False)
    out_d = nc.declare_dram_parameter("out", [1, 3], F32, isOutput=True)

    iota_np = np.tile(np.arange(NUM_CLASSES, dtype=np.float32), (128, 1))
    iota_d = nc.inline_tensor(iota_np, name="iota10")

    with tile.TileContext(nc) as tc:
        with (
            tc.tile_pool(name="data", bufs=2) as dpool,
            tc.tile_pool(name="chunk", bufs=2) as cpool,
            tc.tile_pool(name="small", bufs=2) as spool,
            tc.tile_pool(name="persist", bufs=1) as ppool,
            tc.tile_pool(name="dram", bufs=1, space="DRAM") as drpool,
        ):
            iota_t = ppool.tile([128, NUM_CLASSES], F32)
            nc.sync.dma_start(iota_t[:], iota_d[:])
            # eps constant, produced on DVE so ACT ops reading it alongside
            # rp/rn (also DVE) need only one cross-engine wait
            eps_t = ppool.tile([128, 1], F32)
            nc.vector.tensor_scalar(
                eps_t[:], iota_t[:, 0:1], 0.0, EPS, op0=ALU.mult, op1=ALU.add
            )

            # per-supertile partial columns (persist across the loop)
            klc = ppool.tile([128, ST], F32)
            nzc = ppool.tile([128, ST], F32)
            cec = ppool.tile([128, ST], F32)
            msec = ppool.tile([128, ST], F32)

            for s in range(ST):
                xp_t = dpool.tile([128, pad], F32, tag="xp")
                nc.sync.dma_start(xp_t[:], xp_d[s])
                xn_t = dpool.tile([128, pad], F32, tag="xn")
                nc.sync.dma_start(xn_t[:], xn_d[s])

                # ---- pass 1: per-graph sums (DVE for xp, ACT for xn) ----
                spp = spool.tile([128, nch], F32, tag="spp")
                snp = spool.tile([128, nch], F32, tag="snp")
                for k in range(nch):
                    sl = np.s_[:, k * ch : (k + 1) * ch]
                    nc.vector.reduce_sum(spp[:, k : k + 1], xp_t[sl], axis=AX.X)
                    scr = cpool.tile([128, ch], F32, tag="sn")
                    nc.scalar.activation(
                        scr[:], xn_t[sl], ACTF.Copy, accum_out=snp[:, k : k + 1]
                    )
                sp = spool.tile([128, 1], F32, tag="sp")
                nc.vector.tensor_tensor(sp[:], spp[:, 0:1], spp[:, 1:2], op=ALU.add)
                sn = spool.tile([128, 1], F32, tag="snn")
                nc.vector.tensor_tensor(sn[:], snp[:, 0:1], snp[:, 1:2], op=ALU.add)

                # non-empty graph indicator (counts>0 <=> sum of scores > 0)
                nc.vector.tensor_scalar(
                    nzc[:, s : s + 1], sp[:], 0.0, 0.0, op0=ALU.is_gt, op1=ALU.bypass
                )

                spe = spool.tile([128, 1], F32, tag="spe")
                nc.vector.tensor_scalar(
                    spe[:], sp[:], EPS, 0.0, op0=ALU.add, op1=ALU.bypass
                )
                rp = spool.tile([128, 1], F32, tag="rp")
                nc.vector.reciprocal(rp[:], spe[:])
                sne = spool.tile([128, 1], F32, tag="sne")
                nc.vector.tensor_scalar(
                    sne[:], sn[:], EPS, 0.0, op0=ALU.add, op1=ALU.bypass
                )
                rn = spool.tile([128, 1], F32, tag="rn")
                nc.vector.reciprocal(rn[:], sne[:])

                # ---- pass 2: KL terms ----
                aP = spool.tile([128, nch], F32, tag="aP")
                aN = spool.tile([128, nch], F32, tag="aN")
                aT = spool.tile([128, nch], F32, tag="aT")
                for k in range(nch):
                    sl = np.s_[:, k * ch : (k + 1) * ch]
                    sn_t = cpool.tile([128, ch], F32, tag="sn")
                    nc.scalar.activation(sn_t[:], xn_t[sl], ACTF.Copy, scale=rn[:])
                    lp_t = cpool.tile([128, ch], F32, tag="lp")
                    nc.scalar.activation(
                        lp_t[:], xp_t[sl], ACTF.Ln, bias=eps_t[:], scale=rp[:]
                    )
                    ln_t = cpool.tile([128, ch], F32, tag="ln")
                    nc.scalar.activation(
                        ln_t[:], xn_t[sl], ACTF.Ln, bias=eps_t[:], scale=rn[:]
                    )
                    w_t = cpool.tile([128, ch], F32, tag="w")
                    nc.vector.scalar_tensor_tensor(
                        w_t[:], xp_t[sl], rp[:], sn_t[:], op0=ALU.mult, op1=ALU.add
                    )
                    lm_t = cpool.tile([128, ch], F32, tag="lm")
                    nc.scalar.activation(
                        lm_t[:], w_t[:], ACTF.Ln, bias=eps_t[:], scale=0.5
                    )
                    scr1 = cpool.tile([128, ch], F32, tag="scr")
                    nc.vector.scalar_tensor_tensor(
                        scr1[:], xp_t[sl], 1.0, lp_t[:],
                        op0=ALU.bypass, op1=ALU.mult, accum_out=aP[:, k : k + 1],
                    )
                    scr2 = cpool.tile([128, ch], F32, tag="scr")
                    nc.vector.scalar_tensor_tensor(
                        scr2[:], xn_t[sl], 1.0, ln_t[:],
                        op0=ALU.bypass, op1=ALU.mult, accum_out=aN[:, k : k + 1],
                    )
                    scr3 = cpool.tile([128, ch], F32, tag="scr")
                    nc.vector.scalar_tensor_tensor(
                        scr3[:], w_t[:], 1.0, lm_t[:],
                        op0=ALU.bypass, op1=ALU.mult, accum_out=aT[:, k : k + 1],
                    )

                # klc[:, s] = rp*sum(aP) + rn*sum(aN) - sum(aT)
                aPs = spool.tile([128, 1], F32, tag="aPs")
                nc.vector.tensor_tensor(aPs[:], aP[:, 0:1], aP[:, 1:2], op=ALU.add)
                aNs = spool.tile([128, 1], F32, tag="aNs")
                nc.vector.tensor_tensor(aNs[:], aN[:, 0:1], aN[:, 1:2], op=ALU.add)
                aTs = spool.tile([128, 1], F32, tag="aTs")
                nc.vector.tensor_tensor(aTs[:], aT[:, 0:1], aT[:, 1:2], op=ALU.add)
                t1 = spool.tile([128, 1], F32, tag="t1")
                nc.vector.tensor_tensor(t1[:], aPs[:], rp[:], op=ALU.mult)
                t2 = spool.tile([128, 1], F32, tag="t2")
                nc.vector.scalar_tensor_tensor(
                    t2[:], aNs[:], rn[:], t1[:], op0=ALU.mult, op1=ALU.add
                )
                nc.vector.tensor_tensor(
                    klc[:, s : s + 1], t2[:], aTs[:], op=ALU.subtract
                )

                # ---- CE + MSE for this supertile's 128 graphs ----
                mt_t = spool.tile([128, 32], F32, tag="mt")
                nc.sync.dma_start(mt_t[:], mt_d[s])
                lg = mt_t[:, 0:NUM_CLASSES]
                pp = mt_t[:, NUM_CLASSES : 2 * NUM_CLASSES]
                pn = mt_t[:, 2 * NUM_CLASSES : 3 * NUM_CLASSES]
                tgf = mt_t[:, 30:31]

                mx = spool.tile([128, 1], F32, tag="mx")
                nc.vector.reduce_max(mx[:], lg, axis=AX.X)
                negm = spool.tile([128, 1], F32, tag="negm")
                nc.vector.tensor_scalar(
                    negm[:], mx[:], -1.0, 0.0, op0=ALU.mult, op1=ALU.bypass
                )
                e_t = spool.tile([128, NUM_CLASSES], F32, tag="e")
                nc.scalar.activation(e_t[:], lg, ACTF.Exp, bias=negm[:])
                s1 = spool.tile([128, 1], F32, tag="s1")
                nc.vector.reduce_sum(s1[:], e_t[:], axis=AX.X)
                ls = spool.tile([128, 1], F32, tag="ls")
                nc.scalar.activation(ls[:], s1[:], ACTF.Ln)
                lse = spool.tile([128, 1], F32, tag="lse")
                nc.vector.tensor_tensor(lse[:], ls[:], mx[:], op=ALU.add)
                oh = spool.tile([128, NUM_CLASSES], F32, tag="oh")
                nc.vector.tensor_tensor(
                    oh[:], iota_t[:], tgf.to_broadcast([128, NUM_CLASSES]),
                    op=ALU.is_equal,
                )
                ohs = spool.tile([128, NUM_CLASSES], F32, tag="ohs")
                pick = spool.tile([128, 1], F32, tag="pick")
                nc.vector.scalar_tensor_tensor(
                    ohs[:], oh[:], 1.0, lg, op0=ALU.bypass, op1=ALU.mult,
                    accum_out=pick[:],
                )
                nc.vector.tensor_tensor(
                    cec[:, s : s + 1], lse[:], pick[:], op=ALU.subtract
                )

                d_t = spool.tile([128, NUM_CLASSES], F32, tag="d")
                nc.vector.scalar_tensor_tensor(
                    d_t[:], pp, 1.0, pn, op0=ALU.subtract, op1=ALU.add
                )
                d2_t = spool.tile([128, NUM_CLASSES], F32, tag="d2")
                nc.vector.scalar_tensor_tensor(
                    d2_t[:], d_t[:], 1.0, d_t[:], op0=ALU.bypass, op1=ALU.mult,
                    accum_out=msec[:, s : s + 1],
                )

            # ---- fold the 4 supertile columns, stack into [128, 4] partials ----
            par = ppool.tile([128, 4], F32)
            nc.vector.reduce_sum(par[:, 0:1], klc[:], axis=AX.X)
            nc.vector.reduce_sum(par[:, 1:2], nzc[:], axis=AX.X)
            nc.vector.reduce_sum(par[:, 2:3], cec[:], axis=AX.X)
            nc.vector.reduce_sum(par[:, 3:4], msec[:], axis=AX.X)

            # ---- cross-core AllReduce of the [128,4] partials (via DRAM) ----
            cc_in = drpool.tile([128, 4], F32)
            nc.gpsimd.dma_start(cc_in[:], par[:])
            cc_out = drpool.tile([128, 4], F32)
            nc.gpsimd.collective_compute(
                "AllReduce",
                ALU.add,
                replica_groups=[list(range(NCORES))],
                ins=[cc_in.opt()],
                outs=[cc_out.opt()],
            )
            # reload as [1, 512] and reduce over the partition-index stripes
            allp_t = ppool.tile([1, 512], F32)
            nc.sync.dma_start(
                allp_t[:], cc_out[:].rearrange("p j -> (p j)")[None, :]
            )
            allp4 = ppool.tile([1, 4], F32)
            nc.vector.reduce_sum(
                allp4[:], allp_t[:].rearrange("o (p j) -> o j p", p=128, j=4),
                axis=AX.X,
            )

            # ---- final scalar math (identical on every core) ----
            kl_s = allp4[:, 0:1]
            ng_s = allp4[:, 1:2]
            ce_s = allp4[:, 2:3]
            ms_s = allp4[:, 3:4]

            rng = ppool.tile([1, 1], F32)
            nc.vector.reciprocal(rng[:], ng_s)
            tj = ppool.tile([1, 1], F32)
            nc.vector.tensor_tensor(tj[:], kl_s, rng[:], op=ALU.mult)
            js = ppool.tile([1, 1], F32)
            nc.vector.tensor_scalar(
                js[:], tj[:], 0.5 * ALPHA, 0.0, op0=ALU.mult, op1=ALU.bypass
            )
            lcor = ppool.tile([1, 1], F32)
            nc.vector.scalar_tensor_tensor(
                lcor[:], ms_s, BETA / (NUM_GRAPHS * NUM_CLASSES), js[:],
                op0=ALU.mult, op1=ALU.add,
            )
            ltr = ppool.tile([1, 1], F32)
            nc.vector.tensor_scalar(
                ltr[:], ce_s, 1.0 / NUM_GRAPHS, 0.0, op0=ALU.mult, op1=ALU.bypass
            )
            ltot = ppool.tile([1, 1], F32)
            nc.vector.scalar_tensor_tensor(
                ltot[:], lcor[:], LAMBDA_COR, ltr[:], op0=ALU.mult, op1=ALU.add
            )

            outv = ppool.tile([1, 3], F32)
            nc.vector.tensor_copy(outv[:, 0:1], ltot[:])
            nc.vector.tensor_copy(outv[:, 1:2], ltr[:])
            nc.vector.tensor_copy(outv[:, 2:3], lcor[:])
            nc.sync.dma_start(out_d[:], outv[:])

    nc.finalize()
    return nc


def _pack_host(score_pos, score_neg, batch, pad):
    """Group nodes by graph into a zero-padded [NUM_GRAPHS, pad] layout."""
    n = batch.shape[0]
    counts = np.bincount(batch, minlength=NUM_GRAPHS)
    assert counts.max() <= pad, f"graph size {counts.max()} exceeds pad {pad}"
    order = np.argsort(batch, kind="stable")
    bs = batch[order]
    starts = np.zeros(NUM_GRAPHS, np.int64)
    starts[1:] = np.cumsum(counts)[:-1]
    pos = np.arange(n, dtype=np.int64) - starts[bs]
    xp = np.zeros((NUM_GRAPHS, pad), np.float32)
    xn = np.zeros((NUM_GRAPHS, pad), np.float32)
    xp[bs, pos] = np.asarray(score_pos, np.float32)[order]
    xn[bs, pos] = np.asarray(score_neg, np.float32)[order]
    return xp, xn


_NC_CACHE: dict = {}


def kernel(logits_pos, probs_pos, probs_neg, score_pos, score_neg, targets, batch):
    global LAST_RESULTS
    logits_pos = np.asarray(logits_pos, np.float32)
    probs_pos = np.asarray(probs_pos, np.float32)
    probs_neg = np.asarray(probs_neg, np.float32)
    score_pos = np.asarray(score_pos, np.float32)
    score_neg = np.asarray(score_neg, np.float32)
    targets = np.asarray(targets)
    batch = np.asarray(batch)

    # --- host-side sharding: route nodes to the core owning their graph,
    # grouped by graph with zero padding to a fixed stride ---
    xp, xn = _pack_host(score_pos, score_neg, batch, PAD)
    xp_c = xp.reshape(NCORES, ST, 128, PAD)
    xn_c = xn.reshape(NCORES, ST, 128, PAD)
    mt = np.concatenate(
        [
            logits_pos.reshape(NCORES, ST, 128, NUM_CLASSES),
            probs_pos.reshape(NCORES, ST, 128, NUM_CLASSES),
            probs_neg.reshape(NCORES, ST, 128, NUM_CLASSES),
            targets.astype(np.float32).reshape(NCORES, ST, 128, 1),
            np.zeros((NCORES, ST, 128, 1), np.float32),
        ],
        axis=-1,
    )

    key = (PAD, NCH)
    if key not in _NC_CACHE:
        _NC_CACHE[key] = _build_nc(PAD, NCH)
    nc = _NC_CACHE[key]

    in_maps = [
        {"xp": xp_c[c], "xn": xn_c[c], "mt": mt[c]} for c in range(NCORES)
    ]
    trace = bool(int(os.environ.get("KERNEL_TRACE", "0")))
    res = run_bass_kernel_spmd(nc, in_maps, list(range(NCORES)), trace=trace)
    LAST_RESULTS = res
    out = np.asarray(res.results[0]["out"], np.float32).reshape(3)
    return (np.float32(out[0]), np.float32(out[1]), np.float32(out[2]))


# revision 15
# speedup vs baseline: 1.0882x; 1.0882x over previous
"""Trainium2 Bass kernel for nn_MGCNLoss (segment_reduce).

Strategy (8 NeuronCores, SPMD):
  * Graph-sharded data parallelism: core c owns graphs [512c, 512(c+1)).
  * Host-side sharding step routes every node to its owning core and lays the
    core's nodes out as a fixed-stride padded matrix [512 graphs, PAD slots]
    (zero padding; PAD=2304 >= max nodes/graph). With that layout the on-device
    segment_sum is a dense per-partition row reduction (partition p of
    supertile s holds graph 512c+128s+p), the per-node normalization
    score/(sum[batch]+eps) is a per-partition broadcast, and the whole kernel
    is memory/DVE-bound as the problem's target_regime intends.
  * Device computes, per core: per-graph sums (segment_sum partials), their
    reciprocals, the per-node JS/KL terms (via ACT Ln + DVE fused
    multiply-accumulate), per-graph cross-entropy (max/exp/sum/log-softmax +
    one-hot target pick) and the correlation MSE, reduced to per-partition
    partials; partials are all-reduced across the 8 cores with a collective
    and every core computes the identical final (l_total, l_train, l_cor).

KL identity used (exactly the reference math, no approximation):
    sum_i [s_p*log((s_p+e)/(m+e)) + s_n*log((s_n+e)/(m+e))]
  = sum_i [s_p*Lp + s_n*Ln - (s_p+s_n)*Lm]
  with Lp=log(s_p+e), Ln=log(s_n+e), Lm=log(0.5*(s_p+s_n)+e)
  and sum_i s_p*Lp = r_p * sum_i x_i*Lp  (r_p is constant per graph/partition).
"""

import os

import numpy as np

import concourse.bass as bass
import concourse.bacc as bacc
import concourse.mybir as mybir
from concourse import tile
from concourse.bass_utils import run_bass_kernel_spmd

F32 = mybir.dt.float32
F16 = mybir.dt.float16
ALU = mybir.AluOpType
ACTF = mybir.ActivationFunctionType
AX = mybir.AxisListType

NUM_GRAPHS = 4096
NUM_NODES = 8_388_608
NUM_CLASSES = 10
NCORES = 8
GPC = NUM_GRAPHS // NCORES  # graphs per core = 512
ST = GPC // 128  # supertiles per core = 4
PAD = 2304  # padded slots per graph (actual max graph size is 2229)
NCH = 2  # chunks per supertile for pass 2
EPS = 1e-8
ALPHA = 1.0
BETA = 1.0
LAMBDA_COR = 0.1

LAST_RESULTS = None  # BassKernelResults of the most recent run (for test harness)


def _build_nc(pad: int, nch: int) -> bass.Bass:
    """Build the SPMD Bass program (identical on all 8 cores)."""
    ch = pad // nch
    assert ch * nch == pad
    nc = bacc.Bacc(None, num_devices=NCORES)

    xp_d = nc.declare_dram_parameter("xp", [ST, 128, pad], F32, isOutput=False)
    xn_d = nc.declare_dram_parameter("xn", [ST, 128, pad], F32, isOutput=False)
    # meta: per graph row: [0:10]=logits, [10:20]=probs_pos, [20:30]=probs_neg,
    # [30]=target (as f32), [31]=zero pad
    mt_d = nc.declare_dram_parameter("mt", [ST, 128, 32], F32, isOutput=False)
    out_d = nc.declare_dram_parameter("out", [1, 3], F32, isOutput=True)

    iota_np = np.tile(np.arange(NUM_CLASSES, dtype=np.float32), (128, 1))
    iota_d = nc.inline_tensor(iota_np, name="iota10")

    with tile.TileContext(nc) as tc:
        with (
            tc.tile_pool(name="data", bufs=2) as dpool,
            tc.tile_pool(name="chunk", bufs=2) as cpool,
            tc.tile_pool(name="small", bufs=2) as spool,
            tc.tile_pool(name="persist", bufs=1) as ppool,
            tc.tile_pool(name="dram", bufs=1, space="DRAM") as drpool,
        ):
            iota_t = ppool.tile([128, NUM_CLASSES], F32)
            nc.sync.dma_start(iota_t[:], iota_d[:])
            # eps constant, produced on DVE so ACT ops reading it alongside
            # rp/rn (also DVE) need only one cross-engine wait
            eps_t = ppool.tile([128, 1], F32)
            nc.vector.tensor_scalar(
                eps_t[:], iota_t[:, 0:1], 0.0, EPS, op0=ALU.mult, op1=ALU.add
            )

            # warm-up collective: syncs the 8 cores early (absorbing launch
            # skew) and warms the collectives firmware path, overlapped with
            # the first supertile's compute. Garbage payload, result unused.
            wu_in = drpool.tile([1, 4], F32)
            wu_out = drpool.tile([1, 4], F32)
            wu_s = spool.tile([1, 4], F32, tag="wu")
            nc.gpsimd.memset(wu_s[:], 0.0)
            nc.gpsimd.dma_start(wu_in[:], wu_s[:])
            wu_cc = nc.gpsimd.collective_compute(
                "AllReduce",
                ALU.add,
                replica_groups=[list(range(NCORES))],
                ins=[wu_in.opt()],
                outs=[wu_out.opt()],
            )

            # per-supertile partial columns (persist across the loop)
            klc = ppool.tile([128, ST], F32)
            nzc = ppool.tile([128, ST], F32)
            cec = ppool.tile([128, ST], F32)
            msec = ppool.tile([128, ST], F32)

            for s in range(ST):
                # split each load in halves so pass-1 starts on the first half
                xp_t = dpool.tile([128, pad], F32, tag="xp")
                xn_t = dpool.tile([128, pad], F32, tag="xn")
                hf = pad // 2
                nc.sync.dma_start(xn_t[:, :hf], xn_d[s][:, :hf])
                nc.sync.dma_start(xp_t[:, :hf], xp_d[s][:, :hf])
                nc.sync.dma_start(xn_t[:, hf:], xn_d[s][:, hf:])
                nc.sync.dma_start(xp_t[:, hf:], xp_d[s][:, hf:])

                # ---- pass 1: per-graph sums (both on ACT copy-accum; the
                # fp16 copy outputs land in lp/ln and are overwritten by the
                # Ln activations below, same engine so just program order) ----
                lp_t = cpool.tile([128, pad], F16, tag="lp16")
                ln_t = cpool.tile([128, pad], F16, tag="ln16")
                spp = spool.tile([128, 2], F32, tag="spp")
                snp = spool.tile([128, 2], F32, tag="snp")
                for k in range(2):
                    sl = np.s_[:, k * hf : (k + 1) * hf]
                    _bar_i = nc.scalar.activation(
                        lp_t[sl], xp_t[sl], ACTF.Copy, accum_out=spp[:, k : k + 1]
                    )
                    nc.scalar.activation(
                        ln_t[sl], xn_t[sl], ACTF.Copy, accum_out=snp[:, k : k + 1]
                    )
                # (warmup barrier dep removed - caused exec failure?)
                sp = spool.tile([128, 1], F32, tag="sp")
                nc.vector.tensor_tensor(sp[:], spp[:, 0:1], spp[:, 1:2], op=ALU.add)
                sn = spool.tile([128, 1], F32, tag="snn")
                nc.vector.tensor_tensor(sn[:], snp[:, 0:1], snp[:, 1:2], op=ALU.add)

                # non-empty graph indicator (counts>0 <=> sum of scores > 0)
                nc.vector.tensor_scalar(
                    nzc[:, s : s + 1], sp[:], 0.0, 0.0, op0=ALU.is_gt, op1=ALU.bypass
                )

                spe = spool.tile([128, 1], F32, tag="spe")
                nc.vector.tensor_scalar(
                    spe[:], sp[:], EPS, 0.0, op0=ALU.add, op1=ALU.bypass
                )
                rp = spool.tile([128, 1], F32, tag="rp")
                nc.vector.reciprocal(rp[:], spe[:])
                sne = spool.tile([128, 1], F32, tag="sne")
                nc.vector.tensor_scalar(
                    sne[:], sn[:], EPS, 0.0, op0=ALU.add, op1=ALU.bypass
                )
                rn = spool.tile([128, 1], F32, tag="rn")
                nc.vector.reciprocal(rn[:], sne[:])

                # ---- pass 2: KL terms ----
                # sn on GPSIMD; w via fused affine_then_add; the three
                # product-sums via fused affine_mul_reduce / tensor_tensor_
                # reduce (sp is never materialised - its scale rides the op)
                aPs = spool.tile([128, 1], F32, tag="aPs")
                aNs = spool.tile([128, 1], F32, tag="aNs")
                aTs = spool.tile([128, 1], F32, tag="aTs")

                sn_t = cpool.tile([128, pad], F16, tag="sn16")
                nc.vector.tensor_scalar(
                    sn_t[:], xn_t[:], rn[:], 0.0, op0=ALU.mult, op1=ALU.bypass
                )
                w_t = cpool.tile([128, pad], F16, tag="w16")
                nc.vector.affine_then_add(
                    w_t[:], xp_t[:], sn_t[:], scale=rp[:], bias=0.0
                )
                nc.scalar.activation(
                    lp_t[:], xp_t[:], ACTF.Ln, bias=eps_t[:], scale=rp[:]
                )
                nc.scalar.activation(
                    ln_t[:], xn_t[:], ACTF.Ln, bias=eps_t[:], scale=rn[:]
                )
                lm_t = cpool.tile([128, pad], F16, tag="lm16")
                nc.scalar.activation(
                    lm_t[:], w_t[:], ACTF.Ln, bias=eps_t[:], scale=0.5
                )
                scr_t = cpool.tile([128, pad], F16, tag="scr16")
                nc.vector.affine_mul_reduce(
                    scr_t[:], aPs[:], xp_t[:], lp_t[:], scale=rp[:], bias=0.0
                )
                scr2_t = cpool.tile([128, pad], F16, tag="scr16")
                nc.vector.affine_mul_reduce(
                    scr2_t[:], aNs[:], sn_t[:], ln_t[:], scale=1.0, bias=0.0
                )
                scr3_t = cpool.tile([128, pad], F16, tag="scr16")
                nc.vector.affine_mul_reduce(
                    scr3_t[:], aTs[:], w_t[:], lm_t[:], scale=1.0, bias=0.0
                )

                # klc[:, s] = aPs + aNs - aTs
                t2 = spool.tile([128, 1], F32, tag="t2")
                nc.vector.tensor_tensor(t2[:], aPs[:], aNs[:], op=ALU.add)
                nc.vector.tensor_tensor(
                    klc[:, s : s + 1], t2[:], aTs[:], op=ALU.subtract
                )

                # ---- CE + MSE for this supertile's 128 graphs ----
                mt_t = spool.tile([128, 32], F32, tag="mt")
                nc.sync.dma_start(mt_t[:], mt_d[s])
                lg = mt_t[:, 0:NUM_CLASSES]
                pp = mt_t[:, NUM_CLASSES : 2 * NUM_CLASSES]
                pn = mt_t[:, 2 * NUM_CLASSES : 3 * NUM_CLASSES]
                tgf = mt_t[:, 30:31]

                mx = spool.tile([128, 1], F32, tag="mx")
                nc.vector.reduce_max(mx[:], lg, axis=AX.X)
                negm = spool.tile([128, 1], F32, tag="negm")
                nc.vector.tensor_scalar(
                    negm[:], mx[:], -1.0, 0.0, op0=ALU.mult, op1=ALU.bypass
                )
                e_t = spool.tile([128, NUM_CLASSES], F32, tag="e")
                nc.scalar.activation(e_t[:], lg, ACTF.Exp, bias=negm[:])
                s1 = spool.tile([128, 1], F32, tag="s1")
                nc.vector.reduce_sum(s1[:], e_t[:], axis=AX.X)
                ls = spool.tile([128, 1], F32, tag="ls")
                nc.scalar.activation(ls[:], s1[:], ACTF.Ln)
                lse = spool.tile([128, 1], F32, tag="lse")
                nc.vector.tensor_tensor(lse[:], ls[:], mx[:], op=ALU.add)
                oh = spool.tile([128, NUM_CLASSES], F32, tag="oh")
                nc.vector.tensor_tensor(
                    oh[:], iota_t[:], tgf.to_broadcast([128, NUM_CLASSES]),
                    op=ALU.is_equal,
                )
                ohs = spool.tile([128, NUM_CLASSES], F32, tag="ohs")
                pick = spool.tile([128, 1], F32, tag="pick")
                nc.vector.scalar_tensor_tensor(
                    ohs[:], oh[:], 1.0, lg, op0=ALU.bypass, op1=ALU.mult,
                    accum_out=pick[:],
                )
                nc.vector.tensor_tensor(
                    cec[:, s : s + 1], lse[:], pick[:], op=ALU.subtract
                )

                d_t = spool.tile([128, NUM_CLASSES], F32, tag="d")
                nc.vector.scalar_tensor_tensor(
                    d_t[:], pp, 1.0, pn, op0=ALU.subtract, op1=ALU.add
                )
                d2_t = spool.tile([128, NUM_CLASSES], F32, tag="d2")
                nc.vector.scalar_tensor_tensor(
                    d2_t[:], d_t[:], 1.0, d_t[:], op0=ALU.bypass, op1=ALU.mult,
                    accum_out=msec[:, s : s + 1],
                )

            # ---- fold the 4 supertile columns, stack into [128, 4] partials ----
            par = ppool.tile([128, 4], F32)
            nc.vector.reduce_sum(par[:, 0:1], klc[:], axis=AX.X)
            nc.vector.reduce_sum(par[:, 1:2], nzc[:], axis=AX.X)
            nc.vector.reduce_sum(par[:, 2:3], cec[:], axis=AX.X)
            nc.vector.reduce_sum(par[:, 3:4], msec[:], axis=AX.X)

            # ---- cross-core AllReduce of the [128,4] partials (via DRAM) ----
            cc_in = drpool.tile([128, 4], F32)
            nc.gpsimd.dma_start(cc_in[:], par[:])
            cc_out = drpool.tile([128, 4], F32)
            nc.gpsimd.collective_compute(
                "AllReduce",
                ALU.add,
                replica_groups=[list(range(NCORES))],
                ins=[cc_in.opt()],
                outs=[cc_out.opt()],
            )
            # reload as [1, 512] and reduce over the partition-index stripes
            allp_t = ppool.tile([1, 512], F32)
            nc.sync.dma_start(
                allp_t[:], cc_out[:].rearrange("p j -> (p j)")[None, :]
            )
            allp4 = ppool.tile([1, 4], F32)
            nc.vector.reduce_sum(
                allp4[:], allp_t[:].rearrange("o (p j) -> o j p", p=128, j=4),
                axis=AX.X,
            )

            # ---- final scalar math (identical on every core) ----
            kl_s = allp4[:, 0:1]
            ng_s = allp4[:, 1:2]
            ce_s = allp4[:, 2:3]
            ms_s = allp4[:, 3:4]

            rng = ppool.tile([1, 1], F32)
            nc.vector.reciprocal(rng[:], ng_s)
            tj = ppool.tile([1, 1], F32)
            nc.vector.tensor_tensor(tj[:], kl_s, rng[:], op=ALU.mult)
            js = ppool.tile([1, 1], F32)
            nc.vector.tensor_scalar(
                js[:], tj[:], 0.5 * ALPHA, 0.0, op0=ALU.mult, op1=ALU.bypass
            )
            lcor = ppool.tile([1, 1], F32)
            nc.vector.scalar_tensor_tensor(
                lcor[:], ms_s, BETA / (NUM_GRAPHS * NUM_CLASSES), js[:],
                op0=ALU.mult, op1=ALU.add,
            )
            ltr = ppool.tile([1, 1], F32)
            nc.vector.tensor_scalar(
                ltr[:], ce_s, 1.0 / NUM_GRAPHS, 0.0, op0=ALU.mult, op1=ALU.bypass
            )
            ltot = ppool.tile([1, 1], F32)
            nc.vector.scalar_tensor_tensor(
                ltot[:], lcor[:], LAMBDA_COR, ltr[:], op0=ALU.mult, op1=ALU.add
            )

            outv = ppool.tile([1, 3], F32)
            nc.vector.tensor_copy(outv[:, 0:1], ltot[:])
            nc.vector.tensor_copy(outv[:, 1:2], ltr[:])
            nc.vector.tensor_copy(outv[:, 2:3], lcor[:])
            nc.sync.dma_start(out_d[:], outv[:])

    nc.finalize()
    return nc


def _pack_host(score_pos, score_neg, batch, pad):
    """Group nodes by graph into a zero-padded [NUM_GRAPHS, pad] layout."""
    n = batch.shape[0]
    counts = np.bincount(batch, minlength=NUM_GRAPHS)
    assert counts.max() <= pad, f"graph size {counts.max()} exceeds pad {pad}"
    order = np.argsort(batch, kind="stable")
    bs = batch[order]
    starts = np.zeros(NUM_GRAPHS, np.int64)
    starts[1:] = np.cumsum(counts)[:-1]
    pos = np.arange(n, dtype=np.int64) - starts[bs]
    xp = np.zeros((NUM_GRAPHS, pad), np.float32)
    xn = np.zeros((NUM_GRAPHS, pad), np.float32)
    xp[bs, pos] = np.asarray(score_pos, np.float32)[order]
    xn[bs, pos] = np.asarray(score_neg, np.float32)[order]
    return xp, xn


_NC_CACHE: dict = {}


def kernel(logits_pos, probs_pos, probs_neg, score_pos, score_neg, targets, batch):
    global LAST_RESULTS
    logits_pos = np.asarray(logits_pos, np.float32)
    probs_pos = np.asarray(probs_pos, np.float32)
    probs_neg = np.asarray(probs_neg, np.float32)
    score_pos = np.asarray(score_pos, np.float32)
    score_neg = np.asarray(score_neg, np.float32)
    targets = np.asarray(targets)
    batch = np.asarray(batch)

    # --- host-side sharding: route nodes to the core owning their graph,
    # grouped by graph with zero padding to a fixed stride ---
    xp, xn = _pack_host(score_pos, score_neg, batch, PAD)
    xp_c = xp.reshape(NCORES, ST, 128, PAD)
    xn_c = xn.reshape(NCORES, ST, 128, PAD)
    mt = np.concatenate(
        [
            logits_pos.reshape(NCORES, ST, 128, NUM_CLASSES),
            probs_pos.reshape(NCORES, ST, 128, NUM_CLASSES),
            probs_neg.reshape(NCORES, ST, 128, NUM_CLASSES),
            targets.astype(np.float32).reshape(NCORES, ST, 128, 1),
            np.zeros((NCORES, ST, 128, 1), np.float32),
        ],
        axis=-1,
    )

    key = (PAD, NCH)
    if key not in _NC_CACHE:
        _NC_CACHE[key] = _build_nc(PAD, NCH)
    nc = _NC_CACHE[key]

    in_maps = [
        {"xp": xp_c[c], "xn": xn_c[c], "mt": mt[c]} for c in range(NCORES)
    ]
    trace = bool(int(os.environ.get("KERNEL_TRACE", "0")))
    res = run_bass_kernel_spmd(nc, in_maps, list(range(NCORES)), trace=trace)
    LAST_RESULTS = res
    out = np.asarray(res.results[0]["out"], np.float32).reshape(3)
    return (np.float32(out[0]), np.float32(out[1]), np.float32(out[2]))
